# revision 20
# baseline (speedup 1.0000x reference)
"""Trainium2 Bass kernel for nn_Damping_layer: out = kipf_term - lbda[:, None] * input_term.

Sharding: pure row-parallel over the n_nodes axis across 8 NeuronCores
(12500 rows per core), no cross-core communication. The op is pure
elementwise streaming; the three walls, in the order they were hit, are
HBM bytes, HWDGE descriptor generation (~50M desc/s per ring, 128
descriptors per dma_start), and DVE instruction throughput (~396 ns per
128-row group, the per-partition-scalar op shape caps free size at one
256-elem row). Design:

Precision: input_term is sent as int8 with a per-row scale folded with
lbda on host into one fp32 per-row scalar
    a[row] = -lbda[row] * rowmax|input[row]| / 127
so the device still performs the full per-element multiply+add
    out = (x8 * a) + kipf                     (InstTensorScalarPtr)
with x8 int8, kipf/out bf16. End-to-end L2 relative error ~4e-3, well
inside the 2e-2 gate. Bytes per core: 3.2 + 6.4 + 6.4 = 16 MB.

Layout: 12544 padded rows = 7 tiles of [128 partitions x 14 rows]; per
(tile, partition) the host packs 14 int8 rows (3584 B) then 14 bf16
kipf rows (7168 B), so a full-tile load is ONE dma_start of 128
contiguous 10752-B descriptors. Work is 8 chunks (j-units 7,7,14x6).
The two ramp sub-chunks split their x/k halves across the two HWDGE
rings so descriptor generation overlaps and the DVE starts ~12 us in;
full-tile loads then alternate rings (gen runs far ahead of the
DVE-paced pipeline), and stores alternate opposite so each ring carries
exactly half the load and half the store bytes and no store, which
gates on compute, ever delays a load the DVE is waiting for.

Compute: one scalar_tensor_tensor per 128-row group; the first
GP_PER_TILE groups of each tile run on GpSimd (Pool) instead of DVE,
trimming the DVE critical path; GpSimd always finishes its share first
so stores still gate only on DVE. The per-row scalars ride SWDGE in
the preamble shadow.
"""

import numpy as np

N_NODES = 100000
N_FEAT = 256
N_CORES = 8
ROWS_PER_CORE = N_NODES // N_CORES  # 12500

R_PP = 14                       # rows per partition in a tile
TILE_ROWS = 128 * R_PP          # 1792 rows per tile
N_TILES = 7                     # tiles per core
PAD_ROWS = N_TILES * TILE_ROWS  # 12544 rows per core after padding
LB_COLS = N_TILES * R_PP        # 98
XB = N_FEAT                     # int8 bytes per input row
KB = 2 * N_FEAT                 # bf16 bytes per kipf row
ROWB = XB + KB                  # 768 packed bytes per row
KOFF_B = R_PP * XB              # kipf byte offset within a packed tile line
N_BUFS = 8

_CACHE = {}


def _build_nc():
    from contextlib import ExitStack

    import concourse.bacc as bacc
    import concourse.mybir as mybir
    import concourse.tile as tile

    FP32 = mybir.dt.float32
    BF16 = mybir.dt.bfloat16
    I8 = mybir.dt.int8
    nc = bacc.Bacc(
        "TRN2", target_bir_lowering=False, debug=False, num_devices=N_CORES
    )
    z = nc.dram_tensor(
        "z", [N_TILES * 128, R_PP * ROWB], I8, kind="ExternalInput"
    ).ap()
    al = nc.dram_tensor("al", [128, LB_COLS], FP32, kind="ExternalInput").ap()
    o = nc.dram_tensor("o", [PAD_ROWS, N_FEAT], BF16, kind="ExternalOutput").ap()

    zv = z.rearrange("(t p) b -> t p b", t=N_TILES, p=128)
    ov = o.rearrange("(t p j) c -> t p (j c)", t=N_TILES, p=128, j=R_PP)

    MULT = mybir.AluOpType.mult
    ADD = mybir.AluOpType.add

    with tile.TileContext(nc) as tc, ExitStack() as ctx:
        const = ctx.enter_context(tc.tile_pool(name="const", bufs=1))
        zpool = ctx.enter_context(tc.tile_pool(name="zp", bufs=N_BUFS))
        opool = ctx.enter_context(tc.tile_pool(name="op", bufs=N_BUFS))
        tpool = ctx.enter_context(tc.tile_pool(name="tp", bufs=4))
        COPY = mybir.ActivationFunctionType.Copy

        # per-row fused scalars ride SWDGE (gpsimd), keeping both HWDGE
        # rings' heads free for the first data loads.
        alt = const.tile([128, LB_COLS], FP32)
        nc.gpsimd.dma_start(out=alt[:], in_=al[:])

        # Work list in j-units: [7,7, 14,14,14,14,14, 7,7] = 98.
        # Ring plan (only SP and ACT have HWDGE rings): ALL loads ride
        # the ACT ring -- load dma_start issues are ungated (~0.7 us
        # each on the ACT sequencer stream, between its compute ops) --
        # except chunk 0's x-half, which rides sync so the two ramp
        # loads' serial ~2.56 us descriptor generations overlap. ALL
        # stores ride sync: stores gate on compute, and their semaphore
        # waits must not sit in the ACT sequencer's stream blocking its
        # multiplies.
        chunks = [(0, 0, 7), (0, 7, 14)]
        chunks += [(t, 0, R_PP) for t in range(1, N_TILES - 1)]
        chunks += [(N_TILES - 1, 0, 7), (N_TILES - 1, 7, 14)]

        def emit_load(i):
            t, jlo, jhi = chunks[i]
            nj = jhi - jlo
            zt = zpool.tile([128, R_PP * ROWB], I8, tag="zt")
            if nj == R_PP:
                # whole tile: one load, 128 descriptors of 10752 B
                nc.scalar.dma_start(out=zt[:], in_=zv[t])
            else:
                xeng = nc.sync if i == 0 else nc.scalar
                xeng.dma_start(
                    out=zt[:, jlo * XB : jhi * XB],
                    in_=zv[t][:, jlo * XB : jhi * XB],
                )
                nc.scalar.dma_start(
                    out=zt[:, KOFF_B + jlo * KB : KOFF_B + jhi * KB],
                    in_=zv[t][:, KOFF_B + jlo * KB : KOFF_B + jhi * KB],
                )
            return zt

        def emit_compute_store(i, zt):
            # Compute split: the DVE's fused scalar_tensor_tensor costs
            # ~396 ns per 128-row group (183 wire + ~210 fixed); ACT's
            # per-partition-scale Copy costs ~506 ns. The leading nb
            # groups of each chunk go tmp = x8 * a on ACT, then ONE
            # 2x-packed all-bf16 tensor_tensor add on the DVE covers
            # them (91 ns/group wire + one ~200 ns overhead); trailing
            # groups stay fused on the DVE. Both engines run ~3-4 us
            # per full tile, overlapped.
            t, jlo, jhi = chunks[i]
            nj = jhi - jlo
            nb = nj // 2  # 7 of 14 groups via ACT + one bf16 add
            ot = opool.tile([128, R_PP * N_FEAT], BF16, tag="ot")
            tmp = tpool.tile([128, R_PP * N_FEAT], BF16, tag="tmp")
            for j in range(jlo, jlo + nb):
                c = t * R_PP + j
                nc.scalar.activation(
                    out=tmp[:, j * N_FEAT : (j + 1) * N_FEAT],
                    in_=zt[:, j * XB : (j + 1) * XB],
                    func=COPY,
                    scale=alt[:, c : c + 1],
                )
            for j in range(jlo + nb, jhi):
                c = t * R_PP + j
                kview = zt[:, KOFF_B + j * KB : KOFF_B + (j + 1) * KB].bitcast(
                    BF16
                )
                nc.vector.scalar_tensor_tensor(
                    out=ot[:, j * N_FEAT : (j + 1) * N_FEAT],
                    in0=zt[:, j * XB : (j + 1) * XB],
                    scalar=alt[:, c : c + 1],
                    in1=kview,
                    op0=MULT,
                    op1=ADD,
                )
            bs = slice(jlo * N_FEAT, (jlo + nb) * N_FEAT)
            kbig = zt[
                :, KOFF_B + jlo * KB : KOFF_B + (jlo + nb) * KB
            ].bitcast(BF16)
            nc.vector.tensor_tensor(
                out=ot[:, bs], in0=tmp[:, bs], in1=kbig, op=ADD
            )
            cs = slice(jlo * N_FEAT, jhi * N_FEAT)
            nc.sync.dma_start(out=ov[t][:, cs], in_=ot[:, cs])

        # Software-pipelined emission: W chunk-loads run ahead so each
        # ring's stream starts with pure loads and no store (gated on
        # compute) ever head-of-line-blocks a load the DVE needs soon.
        W = 7
        zts = {}
        for i in range(min(W, len(chunks))):
            zts[i] = emit_load(i)
        for i in range(len(chunks)):
            emit_compute_store(i, zts.pop(i))
            if i + W < len(chunks):
                zts[i + W] = emit_load(i + W)

    nc.compile()
    return nc


def _get_nc():
    if "nc" not in _CACHE:
        _CACHE["nc"] = _build_nc()
    return _CACHE["nc"]


def _shuffle_rows(v_core):
    """[PAD_ROWS] -> [128, LB_COLS] with out[p, t*R_PP+j] = v[t*TILE_ROWS + p*R_PP + j]."""
    return np.ascontiguousarray(
        v_core.reshape(N_TILES, 128, R_PP)
        .transpose(1, 0, 2)
        .reshape(128, LB_COLS)
    )


def _make_in_maps(input_term, kipf_term, lbda):
    import ml_dtypes

    bf16 = ml_dtypes.bfloat16
    input_term = np.asarray(input_term, dtype=np.float32)
    kipf_term = np.asarray(kipf_term, dtype=np.float32).astype(bf16)
    lbda = np.asarray(lbda, dtype=np.float32)

    # per-row int8 quantization of input; lbda folded into the scale
    rowmax = np.abs(input_term).max(axis=1)
    si = np.where(rowmax > 0, rowmax, 1.0).astype(np.float32) / 127.0
    x8 = np.clip(np.rint(input_term / si[:, None]), -127, 127).astype(np.int8)
    a = (-lbda * si).astype(np.float32)

    in_maps = []
    for c in range(N_CORES):
        sl = slice(c * ROWS_PER_CORE, (c + 1) * ROWS_PER_CORE)
        xpad = np.zeros((PAD_ROWS, N_FEAT), np.int8)
        xpad[:ROWS_PER_CORE] = x8[sl]
        kpad = np.zeros((PAD_ROWS, N_FEAT), bf16)
        kpad[:ROWS_PER_CORE] = kipf_term[sl]
        apad = np.zeros((PAD_ROWS,), np.float32)
        apad[:ROWS_PER_CORE] = a[sl]

        # pack per (tile, partition): 14 int8 rows then 14 bf16 rows
        xr = xpad.reshape(N_TILES, 128, R_PP * XB).view(np.uint8)
        kr = kpad.reshape(N_TILES, 128, R_PP, N_FEAT).view(np.uint8)
        zc = np.empty((N_TILES, 128, R_PP * ROWB), np.uint8)
        zc[:, :, :KOFF_B] = xr
        zc[:, :, KOFF_B:] = kr.reshape(N_TILES, 128, R_PP * KB)
        in_maps.append(
            {
                "z": zc.reshape(N_TILES * 128, R_PP * ROWB).view(np.int8),
                "al": _shuffle_rows(apad),
            }
        )
    return in_maps


def kernel(input_term, kipf_term, lbda, spar=None, **_unused):
    from concourse.bass_utils import run_bass_kernel_spmd

    nc = _get_nc()
    in_maps = _make_in_maps(input_term, kipf_term, lbda)
    res = run_bass_kernel_spmd(nc, in_maps, list(range(N_CORES))).results
    return np.concatenate(
        [
            np.asarray(res[c]["o"][:ROWS_PER_CORE], dtype=np.float32)
            for c in range(N_CORES)
        ],
        axis=0,
    )


# revision 21
# speedup vs baseline: 1.1074x; 1.1074x over previous
"""Trainium2 Bass kernel for nn_Damping_layer: out = kipf_term - lbda[:, None] * input_term.

Sharding: pure row-parallel over the n_nodes axis across 8 NeuronCores
(12500 rows per core), no cross-core communication. The op is pure
elementwise streaming; the three walls, in the order they were hit, are
HBM bytes, HWDGE descriptor generation (~50M desc/s per ring, 128
descriptors per dma_start), and DVE instruction throughput (~396 ns per
128-row group, the per-partition-scalar op shape caps free size at one
256-elem row). Design:

Precision: input_term is sent as int8 with a per-row scale folded with
lbda on host into one fp32 per-row scalar
    a[row] = -lbda[row] * rowmax|input[row]| / 127
so the device still performs the full per-element multiply+add
    out = (x8 * a) + kipf                     (InstTensorScalarPtr)
with x8 int8, kipf/out bf16. End-to-end L2 relative error ~4e-3, well
inside the 2e-2 gate. Bytes per core: 3.2 + 6.4 + 6.4 = 16 MB.

Layout: 12544 padded rows = 7 tiles of [128 partitions x 14 rows]; per
(tile, partition) the host packs 14 int8 rows (3584 B) then 14 bf16
kipf rows (7168 B), so a full-tile load is ONE dma_start of 128
contiguous 10752-B descriptors. Work is 8 chunks (j-units 7,7,14x6).
The two ramp sub-chunks split their x/k halves across the two HWDGE
rings so descriptor generation overlaps and the DVE starts ~12 us in;
full-tile loads then alternate rings (gen runs far ahead of the
DVE-paced pipeline), and stores alternate opposite so each ring carries
exactly half the load and half the store bytes and no store, which
gates on compute, ever delays a load the DVE is waiting for.

Compute: one scalar_tensor_tensor per 128-row group; the first
GP_PER_TILE groups of each tile run on GpSimd (Pool) instead of DVE,
trimming the DVE critical path; GpSimd always finishes its share first
so stores still gate only on DVE. The per-row scalars ride SWDGE in
the preamble shadow.
"""

import numpy as np

N_NODES = 100000
N_FEAT = 256
N_CORES = 8
ROWS_PER_CORE = N_NODES // N_CORES  # 12500

R_PP = 14                       # rows per partition in a tile
TILE_ROWS = 128 * R_PP          # 1792 rows per tile
N_TILES = 7                     # tiles per core
PAD_ROWS = N_TILES * TILE_ROWS  # 12544 rows per core after padding
LB_COLS = N_TILES * R_PP        # 98
XB = N_FEAT                     # int8 bytes per input row
KB = 2 * N_FEAT                 # bf16 bytes per kipf row
ROWB = XB + KB                  # 768 packed bytes per row
KOFF_B = R_PP * XB              # kipf byte offset within a packed tile line
N_BUFS = 8

_CACHE = {}


def _build_nc():
    from contextlib import ExitStack

    import concourse.bacc as bacc
    import concourse.mybir as mybir
    import concourse.tile as tile

    FP32 = mybir.dt.float32
    BF16 = mybir.dt.bfloat16
    I8 = mybir.dt.int8
    nc = bacc.Bacc(
        "TRN2", target_bir_lowering=False, debug=False, num_devices=N_CORES
    )
    z = nc.dram_tensor(
        "z", [N_TILES * 128, R_PP * ROWB], I8, kind="ExternalInput"
    ).ap()
    al = nc.dram_tensor("al", [128, LB_COLS], FP32, kind="ExternalInput").ap()
    o = nc.dram_tensor("o", [PAD_ROWS, N_FEAT], BF16, kind="ExternalOutput").ap()

    zv = z.rearrange("(t p) b -> t p b", t=N_TILES, p=128)
    ov = o.rearrange("(t p j) c -> t p (j c)", t=N_TILES, p=128, j=R_PP)

    MULT = mybir.AluOpType.mult
    ADD = mybir.AluOpType.add

    with tile.TileContext(nc) as tc, ExitStack() as ctx:
        const = ctx.enter_context(tc.tile_pool(name="const", bufs=1))
        zpool = ctx.enter_context(tc.tile_pool(name="zp", bufs=N_BUFS))
        opool = ctx.enter_context(tc.tile_pool(name="op", bufs=N_BUFS))
        tpool = ctx.enter_context(tc.tile_pool(name="tp", bufs=4))
        COPY = mybir.ActivationFunctionType.Copy

        # per-row fused scalars ride SWDGE (gpsimd), keeping both HWDGE
        # rings' heads free for the first data loads.
        alt = const.tile([128, LB_COLS], FP32)
        nc.gpsimd.dma_start(out=alt[:], in_=al[:])

        # Work list in j-units: [7,7, 14,14,14,14,14, 7,7] = 98.
        # Ring plan (only SP and ACT have HWDGE rings): ALL loads ride
        # the ACT ring -- load dma_start issues are ungated (~0.7 us
        # each on the ACT sequencer stream, between its compute ops) --
        # except chunk 0's x-half, which rides sync so the two ramp
        # loads' serial ~2.56 us descriptor generations overlap. ALL
        # stores ride sync: stores gate on compute, and their semaphore
        # waits must not sit in the ACT sequencer's stream blocking its
        # multiplies.
        chunks = [(0, 0, 7), (0, 7, 14)]
        chunks += [(t, 0, R_PP) for t in range(1, N_TILES - 1)]
        chunks += [(N_TILES - 1, 0, 7), (N_TILES - 1, 7, 14)]

        def emit_load(i):
            t, jlo, jhi = chunks[i]
            nj = jhi - jlo
            zt = zpool.tile([128, R_PP * ROWB], I8, tag="zt")
            if nj == R_PP:
                # whole tile: one load, 128 descriptors of 10752 B
                nc.scalar.dma_start(out=zt[:], in_=zv[t])
            else:
                xeng = nc.sync if i == 0 else nc.scalar
                xeng.dma_start(
                    out=zt[:, jlo * XB : jhi * XB],
                    in_=zv[t][:, jlo * XB : jhi * XB],
                )
                nc.scalar.dma_start(
                    out=zt[:, KOFF_B + jlo * KB : KOFF_B + jhi * KB],
                    in_=zv[t][:, KOFF_B + jlo * KB : KOFF_B + jhi * KB],
                )
            return zt

        def emit_compute_store(i, zt):
            # Compute split: the DVE's fused scalar_tensor_tensor costs
            # ~396 ns per 128-row group (183 wire + ~210 fixed); ACT's
            # per-partition-scale Copy costs ~506 ns. The leading nb
            # groups of each chunk go tmp = x8 * a on ACT, then ONE
            # 2x-packed all-bf16 tensor_tensor add on the DVE covers
            # them (91 ns/group wire + one ~200 ns overhead); trailing
            # groups stay fused on the DVE. Both engines run ~3-4 us
            # per full tile, overlapped.
            t, jlo, jhi = chunks[i]
            nj = jhi - jlo
            nb = nj // 2  # 7 of 14 groups via ACT + one bf16 add
            ot = opool.tile([128, R_PP * N_FEAT], BF16, tag="ot")
            tmp = tpool.tile([128, R_PP * N_FEAT], BF16, tag="tmp")
            for j in range(jlo, jlo + nb):
                c = t * R_PP + j
                nc.scalar.activation(
                    out=tmp[:, j * N_FEAT : (j + 1) * N_FEAT],
                    in_=zt[:, j * XB : (j + 1) * XB],
                    func=COPY,
                    scale=alt[:, c : c + 1],
                )
            for j in range(jlo + nb, jhi):
                c = t * R_PP + j
                kview = zt[:, KOFF_B + j * KB : KOFF_B + (j + 1) * KB].bitcast(
                    BF16
                )
                nc.vector.scalar_tensor_tensor(
                    out=ot[:, j * N_FEAT : (j + 1) * N_FEAT],
                    in0=zt[:, j * XB : (j + 1) * XB],
                    scalar=alt[:, c : c + 1],
                    in1=kview,
                    op0=MULT,
                    op1=ADD,
                )
            bs = slice(jlo * N_FEAT, (jlo + nb) * N_FEAT)
            kbig = zt[
                :, KOFF_B + jlo * KB : KOFF_B + (jlo + nb) * KB
            ].bitcast(BF16)
            nc.vector.tensor_tensor(
                out=ot[:, bs], in0=tmp[:, bs], in1=kbig, op=ADD
            )
            cs = slice(jlo * N_FEAT, jhi * N_FEAT)
            nc.sync.dma_start(out=ov[t][:, cs], in_=ot[:, cs])

        # Software-pipelined emission: W chunk-loads run ahead so each
        # ring's stream starts with pure loads and no store (gated on
        # compute) ever head-of-line-blocks a load the DVE needs soon.
        W = 6
        zts = {}
        for i in range(min(W, len(chunks))):
            zts[i] = emit_load(i)
        for i in range(len(chunks)):
            emit_compute_store(i, zts.pop(i))
            if i + W < len(chunks):
                zts[i + W] = emit_load(i + W)

    nc.compile()
    return nc


def _get_nc():
    if "nc" not in _CACHE:
        _CACHE["nc"] = _build_nc()
    return _CACHE["nc"]


def _shuffle_rows(v_core):
    """[PAD_ROWS] -> [128, LB_COLS] with out[p, t*R_PP+j] = v[t*TILE_ROWS + p*R_PP + j]."""
    return np.ascontiguousarray(
        v_core.reshape(N_TILES, 128, R_PP)
        .transpose(1, 0, 2)
        .reshape(128, LB_COLS)
    )


def _make_in_maps(input_term, kipf_term, lbda):
    import ml_dtypes

    bf16 = ml_dtypes.bfloat16
    input_term = np.asarray(input_term, dtype=np.float32)
    kipf_term = np.asarray(kipf_term, dtype=np.float32).astype(bf16)
    lbda = np.asarray(lbda, dtype=np.float32)

    # per-row int8 quantization of input; lbda folded into the scale
    rowmax = np.abs(input_term).max(axis=1)
    si = np.where(rowmax > 0, rowmax, 1.0).astype(np.float32) / 127.0
    x8 = np.clip(np.rint(input_term / si[:, None]), -127, 127).astype(np.int8)
    a = (-lbda * si).astype(np.float32)

    in_maps = []
    for c in range(N_CORES):
        sl = slice(c * ROWS_PER_CORE, (c + 1) * ROWS_PER_CORE)
        xpad = np.zeros((PAD_ROWS, N_FEAT), np.int8)
        xpad[:ROWS_PER_CORE] = x8[sl]
        kpad = np.zeros((PAD_ROWS, N_FEAT), bf16)
        kpad[:ROWS_PER_CORE] = kipf_term[sl]
        apad = np.zeros((PAD_ROWS,), np.float32)
        apad[:ROWS_PER_CORE] = a[sl]

        # pack per (tile, partition): 14 int8 rows then 14 bf16 rows
        xr = xpad.reshape(N_TILES, 128, R_PP * XB).view(np.uint8)
        kr = kpad.reshape(N_TILES, 128, R_PP, N_FEAT).view(np.uint8)
        zc = np.empty((N_TILES, 128, R_PP * ROWB), np.uint8)
        zc[:, :, :KOFF_B] = xr
        zc[:, :, KOFF_B:] = kr.reshape(N_TILES, 128, R_PP * KB)
        in_maps.append(
            {
                "z": zc.reshape(N_TILES * 128, R_PP * ROWB).view(np.int8),
                "al": _shuffle_rows(apad),
            }
        )
    return in_maps


def kernel(input_term, kipf_term, lbda, spar=None, **_unused):
    from concourse.bass_utils import run_bass_kernel_spmd

    nc = _get_nc()
    in_maps = _make_in_maps(input_term, kipf_term, lbda)
    res = run_bass_kernel_spmd(nc, in_maps, list(range(N_CORES))).results
    return np.concatenate(
        [
            np.asarray(res[c]["o"][:ROWS_PER_CORE], dtype=np.float32)
            for c in range(N_CORES)
        ],
        axis=0,
    )
